# revision 24
# baseline (speedup 1.0000x reference)
"""TBCNN conv-node kernel for Trainium2 (8 NeuronCores, batch-sharded).

Math (derived from the reference, including its faithful-reshape quirk):
  out[b,n,o] = tanh( nodes[b,n,:] @ Wt + Sr[b,n,:] @ Wr + Sl[b,n,:] @ Wl + bias[o] )
    Sr[b,n,:] = sum_c cr[b,n,c] * nodes[b, ch[b,n,c], :]
    Sl[b,n,:] = sum_c cl[b,n,c] * nodes[b, ch[b,n,c], :]
  where Wt/Wr/Wl are rows 0::3 / 1::3 / 2::3 of concat([w_t, w_r, w_l]) (the
  reference reshapes [F,3] -> [3,F] raw), and cr/cl are the eta_r/eta_l
  coefficients (computed on device), both 0 where ch==0 so the zero-row lookup
  semantics hold while gathering from the raw nodes table.

Architecture per core (2 batches):
  1. nodesT (feature-major nodes) via PE transposes.
  2. Projected table Tcat[m] = [nodes@Wr | nodes@Wl] built on PE, written to
     DRAM (rows are 1KB).
  3. dma_gather (4 SWDGE queues round-robin, 1024 idx/call - ucode ring cap)
     fetches per-(node,child) projected rows in (node-octet, child)-partition
     layout.
  4. DVE pre-scales gathered rows by the eta coefficients (crclP interleaved,
     built from the device-computed coefficients via PE transposes).
  5. One PE matmul per 8-node block vs a constant block-diagonal 0/1 matrix
     accumulates children into an o-major psum; the parent term nodes@Wt
     accumulates into the same psum (lhsT = Wt, rhs = nodesT).
  6. ACT applies bias (per-partition in o-major) + tanh; PE transposes back to
     node-major; DMA out.
"""

import numpy as np
from functools import lru_cache

B, N, C, F, O = 16, 2048, 16, 128, 128
NCORES = 8
BPC = B // NCORES  # batches per core
KBLK = 8  # 8-node blocks per gather call (1024 rows; ucode caps ~1024 idxs)
NBLK = N // 8  # 256 blocks per batch
NCHUNK = NBLK // KBLK  # 32 gather calls per batch
NPC = KBLK * 8  # nodes per chunk (64)
RND = 512  # node-columns per output psum round
NT = N // 128  # 16 node tiles per batch


@lru_cache(maxsize=1)
def _build():
    import concourse.bass as bass
    import concourse.bacc as bacc
    import concourse.tile as tile
    from concourse import mybir

    f32 = mybir.dt.float32
    i32 = mybir.dt.int32
    i16 = mybir.dt.int16
    Alu = mybir.AluOpType
    Act = mybir.ActivationFunctionType

    nc = bacc.Bacc("TRN2", target_bir_lowering=False, debug=False,
                   num_devices=NCORES, num_swdge_queues=4)

    nodes_d = nc.dram_tensor("nodes", [BPC, N, F], f32, kind="ExternalInput")
    cht_d = nc.dram_tensor("cht", [BPC, 128, N], i16, kind="ExternalInput")
    chnat_d = nc.dram_tensor("chnat2", [BPC, 128, 256], i32, kind="ExternalInput")
    wt_d = nc.dram_tensor("wt2", [F, O], f32, kind="ExternalInput")
    wr_d = nc.dram_tensor("wr2", [F, O], f32, kind="ExternalInput")
    wl_d = nc.dram_tensor("wl2", [F, O], f32, kind="ExternalInput")
    bc_d = nc.dram_tensor("bcol", [128, 1], f32, kind="ExternalInput")
    id_d = nc.dram_tensor("ident", [128, 128], f32, kind="ExternalInput")
    m8_d = nc.dram_tensor("mask8", [128, 8], f32, kind="ExternalInput")
    ci_d = nc.dram_tensor("ciota2", [128, 256], f32, kind="ExternalInput")
    k0_d = nc.dram_tensor("k0h2", [128, 256], f32, kind="ExternalInput")
    out_d = nc.dram_tensor("out", [BPC, N, O], f32, kind="ExternalOutput")

    with tile.TileContext(nc) as tc:
        with (
            tc.tile_pool(name="const", bufs=1) as cpool,
            tc.tile_pool(name="work", bufs=2) as pool,
            tc.tile_pool(name="gath", bufs=4) as gpool,
            tc.tile_pool(name="gsc", bufs=3) as spool,
            tc.tile_pool(name="perb", bufs=2) as ppool,
            tc.tile_pool(name="dram", bufs=2, space="DRAM") as dpool,
            tc.tile_pool(name="psO", bufs=2, space="PSUM") as psOpool,
            tc.tile_pool(name="psP", bufs=2, space="PSUM") as psPpool,
            tc.tile_pool(name="psT", bufs=2, space="PSUM") as psTpool,
        ):
            # ---------------- constants ----------------
            wt_s = cpool.tile([F, O], f32)
            wr_s = cpool.tile([F, O], f32)
            wl_s = cpool.tile([F, O], f32)
            bc_s = cpool.tile([128, 1], f32)
            id_s = cpool.tile([128, 128], f32)
            m8_s = cpool.tile([128, 8], f32)
            ci_s = cpool.tile([128, 256], f32)
            k0_s = cpool.tile([128, 256], f32)
            nc.sync.dma_start(wt_s[:], wt_d.ap())
            nc.sync.dma_start(wr_s[:], wr_d.ap())
            nc.sync.dma_start(wl_s[:], wl_d.ap())
            nc.sync.dma_start(bc_s[:], bc_d.ap())
            nc.sync.dma_start(id_s[:], id_d.ap())
            nc.sync.dma_start(m8_s[:], m8_d.ap())
            nc.sync.dma_start(ci_s[:], ci_d.ap())
            nc.sync.dma_start(k0_s[:], k0_d.ap())

            for b in range(BPC):
                # ------------- coefficients (natural layout) -------------
                chnat = pool.tile([128, 256], i32)
                nc.sync.dma_start(chnat[:], chnat_d.ap()[b])
                cht = ppool.tile([128, N], i16)
                nc.sync.dma_start(cht[:], cht_d.ap()[b])

                chf = pool.tile([128, 256], f32)
                nc.vector.tensor_copy(chf[:], chnat[:])
                maskc = pool.tile([128, 256], f32)
                nc.vector.tensor_scalar_min(maskc[:], chf[:], 1.0)
                nsib = pool.tile([128, 16], f32)
                nc.vector.reduce_sum(
                    nsib[:],
                    maskc[:].rearrange("p (n c) -> p n c", c=16),
                    axis=mybir.AxisListType.X,
                )
                denom = pool.tile([128, 16], f32)
                nc.vector.tensor_scalar_add(denom[:], nsib[:], -1.0)
                isone = pool.tile([128, 16], f32)
                nc.vector.tensor_scalar(isone[:], nsib[:], 1.0, None, Alu.is_equal)
                safe = pool.tile([128, 16], f32)
                nc.vector.tensor_add(safe[:], denom[:], isone[:])
                recip = pool.tile([128, 16], f32)
                nc.vector.reciprocal(recip[:], safe[:])

                # cr_general = c * maskc / safe_denom
                crg = pool.tile([128, 256], f32)
                nc.vector.tensor_tensor(crg[:], ci_s[:], maskc[:], op=Alu.mult)
                crg2 = pool.tile([128, 256], f32)
                nc.vector.tensor_tensor(
                    crg2[:].rearrange("p (n c) -> p n c", c=16),
                    crg[:].rearrange("p (n c) -> p n c", c=16),
                    recip[:].unsqueeze(2).to_broadcast([128, 16, 16]),
                    op=Alu.mult,
                )
                # singles branch: where num_sib==1, cr = 0.5*[c==0]
                t1 = pool.tile([128, 256], f32)
                nc.vector.tensor_tensor(t1[:], k0_s[:], crg2[:], op=Alu.subtract)
                t2 = pool.tile([128, 256], f32)
                nc.vector.tensor_tensor(
                    t2[:].rearrange("p (n c) -> p n c", c=16),
                    t1[:].rearrange("p (n c) -> p n c", c=16),
                    isone[:].unsqueeze(2).to_broadcast([128, 16, 16]),
                    op=Alu.mult,
                )
                cr = pool.tile([128, 256], f32)
                nc.vector.tensor_add(cr[:], crg2[:], t2[:])
                creff = pool.tile([128, 256], f32)
                nc.vector.tensor_tensor(creff[:], cr[:], maskc[:], op=Alu.mult)
                cleff = pool.tile([128, 256], f32)
                nc.vector.tensor_tensor(cleff[:], maskc[:], creff[:], op=Alu.subtract)

                # ------------- coef transpose into P-layout -------------
                # crclP[p', 2g+j]: j=0 -> crEff, j=1 -> clEff for node g*8+p'//16,
                # child p'%16. crP[:,2p]=T(creff[:, :128])[:,p] etc.
                crclP = ppool.tile([128, 512], f32)
                for (src, joff) in ((creff, 0), (cleff, 1)):
                    for half in range(2):
                        psT = psTpool.tile([128, 128], f32, tag="psT")
                        nc.tensor.transpose(
                            psT[:], src[:, half * 128:(half + 1) * 128], id_s[:]
                        )
                        dst = crclP[:].rearrange("p (q r) -> p q r", r=4)[
                            :, :, 2 * half + joff
                        ]
                        nc.vector.tensor_copy(dst, psT[:])

                # ------------- nodesT via PE transpose -------------
                nodesT = ppool.tile([128, N], f32)
                for t in range(NT):
                    nsb = pool.tile([128, 128], f32)
                    nc.sync.dma_start(
                        nsb[:], nodes_d.ap()[b, t * 128:(t + 1) * 128, :]
                    )
                    psT2 = psTpool.tile([128, 128], f32, tag="psT")
                    nc.tensor.transpose(psT2[:], nsb[:], id_s[:])
                    nc.vector.tensor_copy(nodesT[:, t * 128:(t + 1) * 128], psT2[:])

                # ------------- projected table Tcat = [T@Wr | T@Wl] -------------
                tcat_d = dpool.tile([N, 2 * O], f32)
                for t in range(NT):
                    psP = psPpool.tile([128, 2 * O], f32)
                    nc.tensor.matmul(
                        psP[:, 0:O], lhsT=nodesT[:, t * 128:(t + 1) * 128],
                        rhs=wr_s[:], start=True, stop=True,
                    )
                    nc.tensor.matmul(
                        psP[:, O:2 * O], lhsT=nodesT[:, t * 128:(t + 1) * 128],
                        rhs=wl_s[:], start=True, stop=True,
                    )
                    stg = pool.tile([128, 2 * O], f32)
                    nc.vector.tensor_copy(stg[:], psP[:])
                    nc.sync.dma_start(
                        tcat_d[t * 128:(t + 1) * 128, :], stg[:]
                    )

                # ------------- gather + reduce + parent + tanh -------------
                for rnd in range(N // RND):  # 4 rounds of 512 node-columns
                    psO = psOpool.tile([128, RND], f32)
                    psB = psOpool.tile([128, RND], f32, tag="psB")
                    # parent term: out^T[o, n] for the round's 4 node tiles
                    for tt in range(RND // 128):
                        t = rnd * (RND // 128) + tt
                        nc.tensor.matmul(
                            psB[:, tt * 128:(tt + 1) * 128],
                            lhsT=wt_s[:],
                            rhs=nodesT[:, t * 128:(t + 1) * 128],
                            start=True, stop=True,
                        )
                    # children: 8 gather chunks of 64 nodes each
                    for qq in range(RND // NPC):
                        q = rnd * (RND // NPC) + qq
                        g = gpool.tile([128, KBLK * 256], f32)
                        nc.gpsimd.dma_gather(
                            out_ap=g[:].rearrange("p (g f) -> p g f", f=256),
                            in_ap=tcat_d[:],
                            idxs_ap=cht[:, q * NPC:(q + 1) * NPC],
                            num_idxs=KBLK * 128,
                            num_idxs_reg=KBLK * 128,
                            elem_size=256,
                            queue_num=(b * NCHUNK + q) % 4,
                        )
                        # pre-scale by [cr | cl] and fold r+l halves
                        gs = spool.tile([128, KBLK * 256], f32)
                        nc.vector.tensor_tensor(
                            gs[:].rearrange("p (g j f) -> p g j f", j=2, f=128),
                            g[:].rearrange("p (g j f) -> p g j f", j=2, f=128),
                            crclP[:, q * 2 * KBLK:(q + 1) * 2 * KBLK]
                            .rearrange("p (g j) -> p g j", j=2)
                            .unsqueeze(3)
                            .to_broadcast([128, KBLK, 2, 128]),
                            op=Alu.mult,
                        )
                        gsum = spool.tile([128, KBLK * 128], f32)
                        nc.vector.tensor_add(
                            gsum[:].rearrange("p (g f) -> p g f", f=128),
                            gs[:].rearrange("p (g j f) -> p g j f", j=2, f=128)[
                                :, :, 0, :
                            ],
                            gs[:].rearrange("p (g j f) -> p g j f", j=2, f=128)[
                                :, :, 1, :
                            ],
                        )
                        for gl in range(KBLK):
                            blk = qq * KBLK + gl  # block within round
                            nc.tensor.matmul(
                                psO[:, blk * 8:(blk + 1) * 8],
                                lhsT=gsum[:, gl * 128:(gl + 1) * 128],
                                rhs=m8_s[:],
                                start=True, stop=True,
                            )
                    # combine parent+children, then bias + tanh (o-major;
                    # bias is per-partition) -> SBUF
                    pt_sb = pool.tile([128, RND], f32)
                    nc.scalar.activation(pt_sb[:], psB[:], Act.Copy)
                    osum = pool.tile([128, RND], f32)
                    nc.vector.tensor_add(osum[:], psO[:], pt_sb[:])
                    ot = pool.tile([128, RND], f32)
                    nc.scalar.activation(ot[:], osum[:], Act.Tanh, bias=bc_s[:])
                    # transpose back to node-major and store
                    for tt in range(RND // 128):
                        t = rnd * (RND // 128) + tt
                        psT3 = psTpool.tile([128, 128], f32, tag="psT")
                        nc.tensor.transpose(
                            psT3[:], ot[:, tt * 128:(tt + 1) * 128], id_s[:]
                        )
                        ob = pool.tile([128, 128], f32)
                        nc.vector.tensor_copy(ob[:], psT3[:])
                        nc.sync.dma_start(
                            out_d.ap()[b, t * 128:(t + 1) * 128, :], ob[:]
                        )

    nc.compile()
    return nc


def _host_prep(nodes, children, w_t, w_r, w_l, b_conv):
    nodes = np.ascontiguousarray(np.asarray(nodes, dtype=np.float32))
    children = np.ascontiguousarray(np.asarray(children, dtype=np.int32))
    w_t = np.asarray(w_t, dtype=np.float32)
    w_r = np.asarray(w_r, dtype=np.float32)
    w_l = np.asarray(w_l, dtype=np.float32)
    b_conv = np.asarray(b_conv, dtype=np.float32)

    wflat = np.concatenate([w_t, w_r, w_l], axis=0)  # [3F, O]
    wt2 = np.ascontiguousarray(wflat[0::3])
    wr2 = np.ascontiguousarray(wflat[1::3])
    wl2 = np.ascontiguousarray(wflat[2::3])
    bcol = np.ascontiguousarray(b_conv[:, None])  # [128, 1]
    ident = np.eye(128, dtype=np.float32)
    mask8 = (np.arange(128)[:, None] // 16 == np.arange(8)[None, :]).astype(
        np.float32
    )
    j = np.arange(256)
    ciota = np.tile((j % 16).astype(np.float32)[None, :], (128, 1))
    k0h = np.tile((0.5 * (j % 16 == 0)).astype(np.float32)[None, :], (128, 1))

    in_maps = []
    for core in range(NCORES):
        bs = slice(core * BPC, (core + 1) * BPC)
        ch = children[bs]  # [BPC, N, C]
        # dma_gather indices: children transposed (column n, row c), int16,
        # replicated across the 8 Q7-core 16-partition stripes
        cht = np.ascontiguousarray(
            np.tile(ch.transpose(0, 2, 1).astype(np.int16), (1, 8, 1))
        )  # [BPC, 128, N]
        chnat = np.ascontiguousarray(ch.reshape(BPC, 128, 256))
        in_maps.append(
            {
                "nodes": np.ascontiguousarray(nodes[bs]),
                "cht": cht,
                "chnat2": chnat,
                "wt2": wt2,
                "wr2": wr2,
                "wl2": wl2,
                "bcol": bcol,
                "ident": ident,
                "mask8": mask8,
                "ciota2": ciota,
                "k0h2": k0h,
            }
        )
    return in_maps


def _run(inputs, trace=False):
    from concourse.bass_utils import run_bass_kernel_spmd

    nc = _build()
    in_maps = _host_prep(
        inputs["nodes"], inputs["children"], inputs["w_t"], inputs["w_r"],
        inputs["w_l"], inputs["b_conv"],
    )
    res = run_bass_kernel_spmd(nc, in_maps, list(range(NCORES)), trace=trace)
    out = np.concatenate([r["out"] for r in res.results], axis=0)
    return out.astype(np.float32), res


def kernel(nodes, children, feature_size=None, w_t=None, w_r=None, w_l=None,
           b_conv=None, **_unused):
    out, _ = _run(
        {
            "nodes": nodes,
            "children": children,
            "w_t": w_t,
            "w_r": w_r,
            "w_l": w_l,
            "b_conv": b_conv,
        }
    )
    return out


# revision 25
# speedup vs baseline: 1.5260x; 1.5260x over previous
"""TBCNN conv-node kernel for Trainium2 (8 NeuronCores, batch-sharded).

Math (derived from the reference, including its faithful-reshape quirk):
  out[b,n,o] = tanh( nodes[b,n,:] @ Wt + Sr[b,n,:] @ Wr + Sl[b,n,:] @ Wl + bias[o] )
    Sr[b,n,:] = sum_c cr[b,n,c] * nodes[b, ch[b,n,c], :]
    Sl[b,n,:] = sum_c cl[b,n,c] * nodes[b, ch[b,n,c], :]
  where Wt/Wr/Wl are rows 0::3 / 1::3 / 2::3 of concat([w_t, w_r, w_l]) (the
  reference reshapes [F,3] -> [3,F] raw), and cr/cl are the eta_r/eta_l
  coefficients (computed on device), both forced to 0 where ch==0 so the
  zero-row lookup semantics hold while gathering from the raw nodes table.

Per core (2 batches):
  - dma_gather (4 SWDGE queues round-robin, 1024 indices per call - the ucode
    descriptor ring caps a single call at ~1024) fetches child rows in a
    (node-octet, child) x feature partition layout.
  - Per 8-node block, one PE matmul against a [128,16] block-diagonal
    coefficient matrix (8 eta_r + 8 eta_l columns) reduces children into
    feature-major SrT/SlT.
  - Stage 2 per node tile: 3 accumulated matmuls (parent/right/left) + bias
    add + tanh, interleaved with the gather chunks so the tail stays short.
"""

import numpy as np
from functools import lru_cache

B, N, C, F, O = 16, 2048, 16, 128, 128
NCORES = 8
BPC = B // NCORES  # batches per core
KBLK = 8  # 8-node gather blocks per chunk (KBLK*128 rows per dma_gather)
NBLK = N // 8  # 256 blocks per batch
NCHUNK = NBLK // KBLK  # 32 chunks per batch
NPC = KBLK * 8  # nodes covered per chunk (64)
NT = N // 128  # 16 node tiles per batch


@lru_cache(maxsize=1)
def _build():
    import concourse.bass as bass
    import concourse.bacc as bacc
    import concourse.tile as tile
    from concourse import mybir

    f32 = mybir.dt.float32
    i32 = mybir.dt.int32
    i16 = mybir.dt.int16
    Alu = mybir.AluOpType
    Act = mybir.ActivationFunctionType

    nc = bacc.Bacc("TRN2", target_bir_lowering=False, debug=False,
                   num_devices=NCORES, num_swdge_queues=4)

    nodes_d = nc.dram_tensor("nodes", [BPC, N, F], f32, kind="ExternalInput")
    cht_d = nc.dram_tensor("cht", [BPC, 128, N], i16, kind="ExternalInput")
    chnat_d = nc.dram_tensor("chnat2", [BPC, 128, 256], i32, kind="ExternalInput")
    wt_d = nc.dram_tensor("wt2", [F, O], f32, kind="ExternalInput")
    wr_d = nc.dram_tensor("wr2", [F, O], f32, kind="ExternalInput")
    wl_d = nc.dram_tensor("wl2", [F, O], f32, kind="ExternalInput")
    bb_d = nc.dram_tensor("bbc", [128, O], f32, kind="ExternalInput")
    id_d = nc.dram_tensor("ident", [128, 128], f32, kind="ExternalInput")
    m8_d = nc.dram_tensor("mask8", [128, 8], f32, kind="ExternalInput")
    ci_d = nc.dram_tensor("ciota2", [128, 256], f32, kind="ExternalInput")
    k0_d = nc.dram_tensor("k0h2", [128, 256], f32, kind="ExternalInput")
    out_d = nc.dram_tensor("out", [BPC, N, O], f32, kind="ExternalOutput")

    with tile.TileContext(nc) as tc:
        with (
            tc.tile_pool(name="const", bufs=1) as cpool,
            tc.tile_pool(name="work", bufs=2) as pool,
            tc.tile_pool(name="gath", bufs=8) as gpool,
            tc.tile_pool(name="abuild", bufs=4) as apool,
            tc.tile_pool(name="perb", bufs=2) as ppool,
            tc.tile_pool(name="ps1", bufs=3, space="PSUM") as ps1pool,
            tc.tile_pool(name="ps2", bufs=2, space="PSUM") as ps2pool,
            tc.tile_pool(name="psT", bufs=2, space="PSUM") as psTpool,
        ):
            # ---------------- constants ----------------
            wt_s = cpool.tile([F, O], f32)
            wr_s = cpool.tile([F, O], f32)
            wl_s = cpool.tile([F, O], f32)
            bb_s = cpool.tile([128, O], f32)
            id_s = cpool.tile([128, 128], f32)
            m8_s = cpool.tile([128, 8], f32)
            ci_s = cpool.tile([128, 256], f32)
            k0_s = cpool.tile([128, 256], f32)
            nc.sync.dma_start(wt_s[:], wt_d.ap())
            nc.sync.dma_start(wr_s[:], wr_d.ap())
            nc.sync.dma_start(wl_s[:], wl_d.ap())
            nc.sync.dma_start(bb_s[:], bb_d.ap())
            nc.sync.dma_start(id_s[:], id_d.ap())
            nc.sync.dma_start(m8_s[:], m8_d.ap())
            nc.sync.dma_start(ci_s[:], ci_d.ap())
            nc.sync.dma_start(k0_s[:], k0_d.ap())

            for b in range(BPC):
                # ------------- gather indices in early -------------
                cht = ppool.tile([128, N], i16)
                nc.sync.dma_start(cht[:], cht_d.ap()[b])
                chnat = pool.tile([128, 256], i32)
                nc.sync.dma_start(chnat[:], chnat_d.ap()[b])

                # ------------- coefficients (natural layout) -------------
                chf = pool.tile([128, 256], f32)
                nc.vector.tensor_copy(chf[:], chnat[:])
                maskc = pool.tile([128, 256], f32)
                nc.vector.tensor_scalar_min(maskc[:], chf[:], 1.0)
                nsib = pool.tile([128, 16], f32)
                nc.vector.reduce_sum(
                    nsib[:],
                    maskc[:].rearrange("p (n c) -> p n c", c=16),
                    axis=mybir.AxisListType.X,
                )
                denom = pool.tile([128, 16], f32)
                nc.vector.tensor_scalar_add(denom[:], nsib[:], -1.0)
                isone = pool.tile([128, 16], f32)
                nc.vector.tensor_scalar(isone[:], nsib[:], 1.0, None, Alu.is_equal)
                safe = pool.tile([128, 16], f32)
                nc.vector.tensor_add(safe[:], denom[:], isone[:])
                recip = pool.tile([128, 16], f32)
                nc.vector.reciprocal(recip[:], safe[:])

                crg = pool.tile([128, 256], f32)
                nc.vector.tensor_tensor(crg[:], ci_s[:], maskc[:], op=Alu.mult)
                crg2 = pool.tile([128, 256], f32)
                nc.vector.tensor_tensor(
                    crg2[:].rearrange("p (n c) -> p n c", c=16),
                    crg[:].rearrange("p (n c) -> p n c", c=16),
                    recip[:].unsqueeze(2).to_broadcast([128, 16, 16]),
                    op=Alu.mult,
                )
                t1 = pool.tile([128, 256], f32)
                nc.vector.tensor_tensor(t1[:], k0_s[:], crg2[:], op=Alu.subtract)
                t2 = pool.tile([128, 256], f32)
                nc.vector.tensor_tensor(
                    t2[:].rearrange("p (n c) -> p n c", c=16),
                    t1[:].rearrange("p (n c) -> p n c", c=16),
                    isone[:].unsqueeze(2).to_broadcast([128, 16, 16]),
                    op=Alu.mult,
                )
                cr = pool.tile([128, 256], f32)
                nc.vector.tensor_add(cr[:], crg2[:], t2[:])
                creff = pool.tile([128, 256], f32)
                nc.vector.tensor_tensor(creff[:], cr[:], maskc[:], op=Alu.mult)
                cleff = pool.tile([128, 256], f32)
                nc.vector.tensor_tensor(cleff[:], maskc[:], creff[:], op=Alu.subtract)

                # ------------- coef transpose into P-layout -------------
                # crclP[p', 2g+j]: j=0 -> crEff, j=1 -> clEff for node
                # g*8 + p'//16, child p'%16.
                crclP = ppool.tile([128, 512], f32)
                for (src, joff) in ((creff, 0), (cleff, 1)):
                    for half in range(2):
                        psT = psTpool.tile([128, 128], f32, tag="psT")
                        nc.tensor.transpose(
                            psT[:], src[:, half * 128:(half + 1) * 128], id_s[:]
                        )
                        dst = crclP[:].rearrange("p (q r) -> p q r", r=4)[
                            :, :, 2 * half + joff
                        ]
                        nc.vector.tensor_copy(dst, psT[:])

                # ------------- nodesT via PE transpose -------------
                nodesT = ppool.tile([128, N], f32)
                for t in range(NT):
                    nsb = pool.tile([128, 128], f32)
                    nc.sync.dma_start(
                        nsb[:], nodes_d.ap()[b, t * 128:(t + 1) * 128, :]
                    )
                    psT2 = psTpool.tile([128, 128], f32, tag="psT")
                    nc.tensor.transpose(psT2[:], nsb[:], id_s[:])
                    nc.vector.tensor_copy(nodesT[:, t * 128:(t + 1) * 128], psT2[:])

                # ------------- gather + stage 1 + interleaved stage 2 -------
                srt = ppool.tile([128, N], f32)
                slt = ppool.tile([128, N], f32)
                for q in range(NCHUNK):
                    g = gpool.tile([128, KBLK * 128], f32)
                    nc.gpsimd.dma_gather(
                        out_ap=g[:].rearrange("p (g f) -> p g f", f=128),
                        in_ap=nodes_d.ap()[b],
                        idxs_ap=cht[:, q * NPC:(q + 1) * NPC],
                        num_idxs=KBLK * 128,
                        num_idxs_reg=KBLK * 128,
                        elem_size=128,
                        queue_num=(b * NCHUNK + q) % 4,
                    )
                    aall = apool.tile([128, KBLK * 16], f32)
                    nc.vector.tensor_tensor(
                        aall[:].rearrange("p (g j m) -> p g j m", j=2, m=8),
                        m8_s[:].unsqueeze(1).unsqueeze(1).to_broadcast(
                            [128, KBLK, 2, 8]
                        ),
                        crclP[:, q * 2 * KBLK:(q + 1) * 2 * KBLK]
                        .rearrange("p (g j) -> p g j", j=2)
                        .unsqueeze(3)
                        .to_broadcast([128, KBLK, 2, 8]),
                        op=Alu.mult,
                    )
                    ps1 = ps1pool.tile([128, KBLK * 16], f32)
                    for gl in range(KBLK):
                        nc.tensor.matmul(
                            ps1[:, gl * 16:(gl + 1) * 16],
                            lhsT=g[:, gl * 128:(gl + 1) * 128],
                            rhs=aall[:, gl * 16:(gl + 1) * 16],
                            start=True,
                            stop=True,
                        )
                    nc.vector.tensor_copy(
                        srt[:, q * NPC:(q + 1) * NPC].rearrange(
                            "p (g m) -> p g m", m=8
                        ),
                        ps1[:].rearrange("p (g m) -> p g m", m=16)[:, :, 0:8],
                    )
                    nc.vector.tensor_copy(
                        slt[:, q * NPC:(q + 1) * NPC].rearrange(
                            "p (g m) -> p g m", m=8
                        ),
                        ps1[:].rearrange("p (g m) -> p g m", m=16)[:, :, 8:16],
                    )

                    # ---- stage 2 for the node tile completed by this chunk
                    if q % 2 == 1:
                        t = q // 2
                        ps2 = ps2pool.tile([128, O], f32)
                        nc.tensor.matmul(
                            ps2[:], lhsT=nodesT[:, t * 128:(t + 1) * 128],
                            rhs=wt_s[:], start=True, stop=False,
                        )
                        nc.tensor.matmul(
                            ps2[:], lhsT=srt[:, t * 128:(t + 1) * 128],
                            rhs=wr_s[:], start=False, stop=False,
                        )
                        nc.tensor.matmul(
                            ps2[:], lhsT=slt[:, t * 128:(t + 1) * 128],
                            rhs=wl_s[:], start=False, stop=True,
                        )
                        tmp = pool.tile([128, O], f32)
                        nc.vector.tensor_add(tmp[:], ps2[:], bb_s[:])
                        outsb = pool.tile([128, O], f32)
                        nc.scalar.activation(outsb[:], tmp[:], Act.Tanh)
                        nc.sync.dma_start(
                            out_d.ap()[b, t * 128:(t + 1) * 128, :], outsb[:]
                        )

    nc.compile()
    return nc


def _host_prep(nodes, children, w_t, w_r, w_l, b_conv):
    nodes = np.ascontiguousarray(np.asarray(nodes, dtype=np.float32))
    children = np.ascontiguousarray(np.asarray(children, dtype=np.int32))
    w_t = np.asarray(w_t, dtype=np.float32)
    w_r = np.asarray(w_r, dtype=np.float32)
    w_l = np.asarray(w_l, dtype=np.float32)
    b_conv = np.asarray(b_conv, dtype=np.float32)

    wflat = np.concatenate([w_t, w_r, w_l], axis=0)  # [3F, O]
    wt2 = np.ascontiguousarray(wflat[0::3])
    wr2 = np.ascontiguousarray(wflat[1::3])
    wl2 = np.ascontiguousarray(wflat[2::3])
    bb = np.tile(b_conv[None, :], (128, 1)).astype(np.float32)
    ident = np.eye(128, dtype=np.float32)
    mask8 = (np.arange(128)[:, None] // 16 == np.arange(8)[None, :]).astype(
        np.float32
    )
    j = np.arange(256)
    ciota = np.tile((j % 16).astype(np.float32)[None, :], (128, 1))
    k0h = np.tile((0.5 * (j % 16 == 0)).astype(np.float32)[None, :], (128, 1))

    in_maps = []
    for core in range(NCORES):
        bs = slice(core * BPC, (core + 1) * BPC)
        ch = children[bs]  # [BPC, N, C]
        cht = np.ascontiguousarray(
            np.tile(ch.transpose(0, 2, 1).astype(np.int16), (1, 8, 1))
        )  # [BPC, 128, N]
        chnat = np.ascontiguousarray(ch.reshape(BPC, 128, 256))
        in_maps.append(
            {
                "nodes": np.ascontiguousarray(nodes[bs]),
                "cht": cht,
                "chnat2": chnat,
                "wt2": wt2,
                "wr2": wr2,
                "wl2": wl2,
                "bbc": bb,
                "ident": ident,
                "mask8": mask8,
                "ciota2": ciota,
                "k0h2": k0h,
            }
        )
    return in_maps


def _run(inputs, trace=False):
    from concourse.bass_utils import run_bass_kernel_spmd

    nc = _build()
    in_maps = _host_prep(
        inputs["nodes"], inputs["children"], inputs["w_t"], inputs["w_r"],
        inputs["w_l"], inputs["b_conv"],
    )
    res = run_bass_kernel_spmd(nc, in_maps, list(range(NCORES)), trace=trace)
    out = np.concatenate([r["out"] for r in res.results], axis=0)
    return out.astype(np.float32), res


def kernel(nodes, children, feature_size=None, w_t=None, w_r=None, w_l=None,
           b_conv=None, **_unused):
    out, _ = _run(
        {
            "nodes": nodes,
            "children": children,
            "w_t": w_t,
            "w_r": w_r,
            "w_l": w_l,
            "b_conv": b_conv,
        }
    )
    return out
